# revision 35
# baseline (speedup 1.0000x reference)
"""DigitCaps dynamic-routing kernel for 8 Trainium2 NeuronCores.

Strategy: shard the routes dimension R=1024 across the 8 cores (128 routes
per core).  u_hat = einsum('rcoi,bri->brco') is never materialized: each
routing iteration computes its weighted route-sum

    s[b,c,o] = sum_{r,i} x[b,r,i] * (c_ij[r,c] * W[r,c,o,i])

directly on the PE as 16 accumulating matmuls with the 128 local routes on
the contraction axis.  The per-core partial s is summed across cores with an
AllReduce (iters 0,1) / ReduceScatter (final iter), fp16 payloads.  The
agreement update

    agree[r,c] = (1/B) sum_{b,o,i} W[r,c,o,i] * x[b,r,i] * v[b,c,o]

is computed locally (no collective) via G[r,i,c,o] = sum_b x[b,r,i]*v[b,c,o]
(PE matmuls) followed by a multiply-reduce against W split across the DVE
and Pool engines.  b_ij / softmax live entirely on the owning core's route
shard.

v2 changes vs the first working kernel:
  - x is staged host-side in BOTH layouts (b-major for the agreement
    matmuls, r-major for the route-sum matmuls), so the 16 on-device
    DMA-transposes are gone; startup is a handful of large DMAs.
  - squash's sqrt uses the int bit-hack + one Newton step on DVE, so the
    Act engine's only table function is the softmax Exp: exactly one
    activation-table load per program instead of two reloads (1.3us each)
    per routing iteration.
  - collectives carry fp16; minimal DMA dispatches around each collective.
  - softmax skips the max-subtraction (logits are small; exp is safe).
  - wp / wden element-wise work is split across DVE and Pool.
"""

import sys

for _p in ("/opt/trn_rl_repo",):
    if _p not in sys.path:
        sys.path.insert(0, _p)

import numpy as np

import concourse.bass as bass
import concourse.bacc as bacc
import concourse.mybir as mybir
import concourse.tile as tile
from concourse.bass_utils import run_bass_kernel_spmd

F32 = mybir.dt.float32
BF16 = mybir.dt.float16  # 16-bit staging dtype for matmul operands
USE_BF16 = True

B, R, C, O, I = 256, 1024, 10, 16, 8
NCORES = 8
RS = R // NCORES          # routes per core
CO = C * O                # 160
COI = C * O * I           # 1280
BS = B // NCORES          # output batch shard per core
NITER = 3
A0 = 0.1                  # softmax(0) over C=10 entries
PACE_AR = 24              # pacer matmuls per collective window
PACE_MID = 10             # pacer matmuls per DVE stretch
AF = mybir.ActivationFunctionType
CSPLIT = 7                # capsules 0:CSPLIT on DVE, CSPLIT:C on Pool
                          # (DVE ~1.1ns/col vs Pool ~2.2ns/col -> 7/3)


SQRT_MAGIC = 0x1FBB67A8


def _squash_block(nc, pool, s_sb, v_sb, nb, alpha, suffix, groups=None):
    """v = squash(alpha * s) given unscaled route-sum s (SBUF [p, nb*C*O]).

    gfac = a2*sqrt(q)/(1 + a2*q) with q[p,g] = sum_o s^2 and a2 = alpha^2;
    v = s * gfac, algebraically equal to squash(alpha*s) along the O axis.
    sqrt(q) is the int bit-hack estimate ((q:i32 >> 1) + MAGIC) polished by
    one Newton step (max rel err 6e-4), so the Act engine never needs a
    Sqrt/Ln table and stays parked on the softmax Exp table all kernel.
    """
    p = s_sb.shape[0]
    g = groups if groups is not None else nb * C
    a2 = float(alpha) * float(alpha)
    tmp_sq = pool.tile([p, g * O], F32, name=f"tmp_sq{suffix}", tag="sq_tmp")
    q = pool.tile([p, g], F32, name=f"q{suffix}", tag="sq_q")
    u0 = pool.tile([p, g], F32, name=f"u0{suffix}", tag="sq_u0")
    ru = pool.tile([p, g], F32, name=f"ru{suffix}", tag="sq_ru")
    ha = pool.tile([p, g], F32, name=f"ha{suffix}", tag="sq_ha")
    un = pool.tile([p, g], F32, name=f"un{suffix}", tag="sq_un")
    wden = pool.tile([p, g], F32, name=f"wden{suffix}", tag="sq_w")
    rw = pool.tile([p, g], F32, name=f"rw{suffix}", tag="sq_r")
    gfac = pool.tile([p, g], F32, name=f"gfac{suffix}", tag="sq_g")

    if nb == 2:
        # square+reduce per batch-half: the first half starts while the
        # second half's collective-readback DMA is still in flight
        # (FP-exact: identical elementwise ops and per-group sum order)
        h = g * O // 2
        for b0 in range(2):
            nc.vector.tensor_mul(
                tmp_sq[:, b0 * h : (b0 + 1) * h],
                s_sb[:, b0 * h : (b0 + 1) * h],
                s_sb[:, b0 * h : (b0 + 1) * h],
            )
            nc.vector.tensor_reduce(
                q[:, b0 * g // 2 : (b0 + 1) * g // 2],
                tmp_sq[:, b0 * h : (b0 + 1) * h].rearrange(
                    "p (g o) -> p g o", o=O
                ),
                axis=mybir.AxisListType.X,
                op=mybir.AluOpType.add,
            )
    else:
        nc.vector.tensor_mul(tmp_sq[:], s_sb[:], s_sb[:])
        nc.vector.tensor_reduce(
            q[:],
            tmp_sq[:].rearrange("p (g o) -> p g o", o=O),
            axis=mybir.AxisListType.X,
            op=mybir.AluOpType.add,
        )
    # u0 ~= sqrt(q) via exponent halving; one Newton step -> un
    # (the verifier forbids mixing bitwise and arith ops in one
    # tensor_scalar, so shift and magic-add stay separate)
    nc.vector.tensor_single_scalar(
        u0[:].bitcast(mybir.dt.int32),
        q[:].bitcast(mybir.dt.int32),
        1,
        mybir.AluOpType.logical_shift_right,
    )
    nc.vector.tensor_single_scalar(
        u0[:].bitcast(mybir.dt.int32),
        u0[:].bitcast(mybir.dt.int32),
        SQRT_MAGIC,
        mybir.AluOpType.add,
    )
    nc.vector.reciprocal(ru[:], u0[:])
    nc.vector.scalar_tensor_tensor(
        ha[:], q[:], 0.5, ru[:], mybir.AluOpType.mult, mybir.AluOpType.mult
    )
    nc.vector.scalar_tensor_tensor(
        un[:], u0[:], 0.5, ha[:], mybir.AluOpType.mult, mybir.AluOpType.add
    )
    # wden stays on DVE: a Pool offload costs two cross-engine semaphore
    # hops (~200ns) to save a ~130ns op
    nc.vector.tensor_scalar(
        wden[:], q[:], a2, 1.0, mybir.AluOpType.mult, mybir.AluOpType.add
    )
    nc.vector.reciprocal(rw[:], wden[:])
    nc.vector.scalar_tensor_tensor(
        gfac[:], un[:], a2, rw[:], mybir.AluOpType.mult, mybir.AluOpType.mult
    )
    nc.vector.tensor_mul(
        v_sb[:].rearrange("p (g o) -> p g o", o=O),
        s_sb[:].rearrange("p (g o) -> p g o", o=O),
        gfac[:].unsqueeze(2).broadcast_to((p, g, O)),
    )


def build_nc(reps=1, niter=NITER, fake_cc=False, chain=False, cc16=True,
             split_cc=False, pace=True, use_ag=False,
             pace_ar=PACE_AR, pace_mid=PACE_MID):
    assert not split_cc and not use_ag, "v2 kernel dropped these paths"
    nc = bacc.Bacc(
        "TRN2", target_bir_lowering=False, debug=False, num_devices=NCORES
    )
    DT = BF16 if USE_BF16 else F32
    xs_d = nc.dram_tensor("xs", [B, I * RS], DT, kind="ExternalInput")
    xt_d = nc.dram_tensor("xt", [RS, 16 * 128], DT, kind="ExternalInput")
    ws_d = nc.dram_tensor("ws", [RS, COI], DT, kind="ExternalInput")
    out_d = nc.dram_tensor("vout", [BS, CO], F32, kind="ExternalOutput")

    with tile.TileContext(nc) as tc:
        with (
            tc.tile_pool(name="main", bufs=1) as pool,
            tc.tile_pool(name="ps", bufs=1, space=bass.MemorySpace.PSUM) as ps,
            tc.tile_pool(name="pg", bufs=1, space=bass.MemorySpace.PSUM) as pg,
            tc.tile_pool(name="dram", bufs=1, space="DRAM") as dram,
        ):
            pools = (pool, ps, pg, dram)
            for rep in range(reps):
                _build_body(nc, tc, pools, xs_d, xt_d, ws_d, out_d, rep,
                            niter=niter, fake_cc=fake_cc,
                            chain=chain and rep > 0, cc16=cc16, pace=pace,
                            pace_ar=pace_ar, pace_mid=pace_mid)
    nc.finalize()
    return nc


def _build_body(nc, tc, pools, xs_d, xt_d, ws_d, out_d, rep, niter=NITER,
                fake_cc=False, chain=False, cc16=True, pace=True,
                pace_ar=PACE_AR, pace_mid=PACE_MID):

    def _pace_pe(nc, dummy_ps, x_T, n):
        # Keep the PE p-state high across collective/DVE windows: self-paced
        # throwaway matmuls into a scratch PSUM bank.  No consumers, only
        # read x_T, so they fill PE idle time without delaying ready work by
        # more than their own tail.
        for _ in range(n):
            nc.tensor.matmul(
                dummy_ps[:], x_T[:, 0:128], x_T[:, 0:512],
                start=True, stop=True, skip_group_check=True,
            )

    rg = [list(range(NCORES))]
    pool, ps, pg, dram = pools
    rp = f"r{rep}_"
    DT = BF16 if USE_BF16 else F32
    CCDT = BF16 if cc16 else F32

    # ---------------- load ----------------
    x_sb = pool.tile([128, 2 * I * RS], DT)    # [b%128, bc*1024 + i*128 + r]
    w_sb = pool.tile([RS, COI], DT)            # [r, c*128 + o*8 + i]
    x_T = pool.tile([128, 16 * 128], DT)       # [r, (bc*8+i)*128 + b%128]

    # w/xt first: they have no cross-rep dependency, so they (and the
    # iter-0 matmuls they feed) complete during the previous rep's tail
    # instead of serializing behind the poison below.
    nc.sync.dma_start(w_sb[:], ws_d[:])
    nc.sync.dma_start(x_T[:, 0:1024], xt_d[:, 0:1024])
    nc.sync.dma_start(x_T[:, 1024:2048], xt_d[:, 1024:2048])
    if chain:
        # Serialize this rep behind the previous one's final output: a tiny
        # DMA from out_d into x_sb creates a WAW overlap with the real x
        # load below, so timing reps measure end-to-end latency.
        poison = out_d[0:32, 0:20]
        if x_sb.dtype != F32:
            poison = poison.bitcast(x_sb.dtype)
        nc.sync.dma_start(x_sb[0:32, 0 : poison.shape[1]], poison)
    # bc0's first 512 cols land first so the iter-0 G matmuls (which gate
    # on the poison chain when collectives are fast) start half a load
    # earlier
    nc.sync.dma_start(x_sb[:, 0:512], xs_d[0:128, 0:512])
    nc.sync.dma_start(x_sb[:, 512:1024], xs_d[0:128, 512:1024])
    nc.sync.dma_start(x_sb[:, 1024:2048], xs_d[128:256, :])

    # W viewed [p, c, o, i]
    w4 = w_sb[:].rearrange("p (c o i) -> p c o i", c=C, o=O, i=I)
    # W viewed [p, i, c, o] (for the agree multiply against G)
    w_ico = w_sb[:].rearrange("p (c o i) -> p i c o", c=C, o=O, i=I)
    # W viewed [p, c, oi] (for the wp scaling)
    w3 = w_sb[:].rearrange("p (c oi) -> p c oi", c=C)

    if niter == 0:
        z = pool.tile([BS, CO], F32, name=f"{rp}z0")
        nc.vector.tensor_copy(z[:], x_T[0:BS, 0:CO])
        nc.sync.dma_start(out_d[:], z[:])
        return

    s_ps0 = [
        ps.tile([128, CO], F32, tag=f"s_ps{bc}", name=f"{rp}s_ps{bc}_0")
        for bc in range(2)
    ]
    dummy_ps = (
        ps.tile([128, 512], F32, tag="dummy_ps", name=f"{rp}dummy_ps")
        if pace else None
    )
    for bc in range(2):
        for i in range(I):
            nc.tensor.matmul(
                s_ps0[bc][:],
                x_T[:, (bc * 8 + i) * 128 : (bc * 8 + i + 1) * 128],
                w4[:, :, :, i],
                start=(i == 0),
                stop=(i == I - 1),
            )

    # collective bounce buffers
    cc_in = [dram.tile([B, CO], CCDT, name=f"{rp}cc_in{t}") for t in range(NITER)]
    cc_out = [
        dram.tile([B, CO], CCDT, name=f"{rp}cc_out{t}", addr_space="Shared")
        for t in range(NITER - 1)
    ]
    rs_out = dram.tile([BS, CO], CCDT, name=f"{rp}rs_out")

    # per-iteration scaled weights
    wp = [None] + [
        pool.tile([RS, COI], DT, name=f"{rp}wp{t}", tag=f"wp{t}") for t in (1, 2)
    ]
    b_ij = [None, None, None]

    s_sb = pool.tile([128, 2 * CO], CCDT)
    v_sb = pool.tile([128, 2 * CO], DT)

    for t in range(niter):
        last = t == niter - 1
        # ---- route-weighted sum matmuls ----
        if t == 0:
            s_ps = s_ps0
        else:
            rhs4 = wp[t][:].rearrange("p (c o i) -> p c o i", c=C, o=O, i=I)
            s_ps = [
                ps.tile([128, CO], F32, tag=f"s_ps{bc}", name=f"{rp}s_ps{bc}_{t}")
                for bc in range(2)
            ]
            for bc in range(2):
                for i in range(I):
                    nc.tensor.matmul(
                        s_ps[bc][:],
                        x_T[:, (bc * 8 + i) * 128 : (bc * 8 + i + 1) * 128],
                        rhs4[:, :, :, i],
                        start=(i == 0),
                        stop=(i == I - 1),
                    )
        s_cat = pool.tile([128, 2 * CO], CCDT, tag="s_cat", name=f"{rp}s_cat_{t}")
        # bc0 copy + DMA dispatch hide under the bc1 matmuls
        nc.scalar.copy(s_cat[:, 0:CO], s_ps[0][:])
        nc.sync.dma_start(cc_in[t][0:128, :], s_cat[:, 0:CO])
        nc.vector.tensor_copy(s_cat[:, CO : 2 * CO], s_ps[1][:])
        nc.sync.dma_start(cc_in[t][128:256, :], s_cat[:, CO : 2 * CO])

        if not last:
            if fake_cc:
                src_ar = cc_in[t]
            else:
                nc.gpsimd.collective_compute(
                    "AllReduce",
                    mybir.AluOpType.add,
                    replica_groups=rg,
                    ins=[cc_in[t][:].opt()],
                    outs=[cc_out[t][:].opt()],
                )
                if pace:
                    _pace_pe(nc, dummy_ps, x_T, pace_ar)
                src_ar = cc_out[t]
            # per-half readback so the squash can start on bc0 while bc1
            # is still on the wire
            nc.sync.dma_start(s_sb[:, 0:CO], src_ar[0:128, :])
            nc.sync.dma_start(s_sb[:, CO : 2 * CO], src_ar[128:256, :])

            alpha = A0 if t == 0 else 1.0
            _squash_block(nc, pool, s_sb, v_sb, 2, alpha, f"_{rp}{t}")

            # ---- G[r, i, c, o] = sum_b x[b,r,i] * v[b,c,o] ----
            g_ps = [
                pg.tile([128, 3 * CO], F32, tag=f"g_ps{gg}", name=f"{rp}g_ps{gg}_{t}")
                for gg in range(3)
            ]
            for i in range(I):
                out_ap = g_ps[i // 3][:, (i % 3) * CO : (i % 3 + 1) * CO]
                for bc in range(2):
                    nc.tensor.matmul(
                        out_ap,
                        x_sb[:, bc * 1024 + i * 128 : bc * 1024 + (i + 1) * 128],
                        v_sb[:, bc * CO : (bc + 1) * CO],
                        start=(bc == 0),
                        stop=(bc == 1),
                    )

            if pace:
                _pace_pe(nc, dummy_ps, x_T, pace_mid)
            # ---- agree[r,c] = (1/B) sum_{o,i} W[r,c,o,i] * G[r,i,c,o] ----
            # G lives in PSUM, which GPSIMD cannot read: all on DVE
            tmpA = pool.tile([128, COI], F32, name=f"{rp}tmpA_{t}", tag="tmpA")
            tA4 = tmpA[:].rearrange("p (c o i) -> p i c o", c=C, o=O, i=I)
            for gg in range(3):
                i0 = gg * 3
                ni = 3 if gg < 2 else 2
                gv = g_ps[gg][:, 0 : ni * CO].rearrange(
                    "p (i c o) -> p i c o", i=ni, c=C, o=O
                )
                nc.vector.tensor_mul(
                    tA4[:, i0 : i0 + ni], w_ico[:, i0 : i0 + ni], gv[:]
                )
            agree = pool.tile([128, C], F32, name=f"{rp}agree_{t}", tag="agree_t")
            tAc = tmpA[:].rearrange("p (c oi) -> p c oi", c=C)
            nc.vector.tensor_reduce(
                agree[:], tAc[:],
                axis=mybir.AxisListType.X, op=mybir.AluOpType.add,
            )
            # ---- b_ij update ----
            bnew = pool.tile([RS, C], F32, name=f"{rp}b_ij_{t}", tag=f"b_ij{t}")
            if t == 0:
                nc.vector.tensor_scalar_mul(bnew[:], agree[:], 1.0 / B)
            else:
                nc.vector.scalar_tensor_tensor(
                    bnew[:], agree[:], 1.0 / B, b_ij[t - 1][:],
                    mybir.AluOpType.mult, mybir.AluOpType.add,
                )
            b_ij[t] = bnew

            # ---- c = softmax(b) over C (no max-sub; logits are small) ----
            e_sb = pool.tile([RS, C], F32, name=f"{rp}e_{t}", tag="e_sb")
            se = pool.tile([RS, 1], F32, name=f"{rp}se_{t}", tag="se")
            rse = pool.tile([RS, 1], F32, name=f"{rp}rse_{t}", tag="rse")
            c_mul = pool.tile([RS, C], DT, name=f"{rp}cb_{t}", tag="c_bf")
            nc.scalar.activation(
                e_sb[:], bnew[:], AF.Exp, accum_out=se[:]
            )
            nc.vector.reciprocal(rse[:], se[:])
            nc.vector.tensor_scalar_mul(c_mul[:], e_sb[:], rse[:])
            # ---- wp = c * W, split DVE / Pool ----
            wp3 = wp[t + 1][:].rearrange("p (c oi) -> p c oi", c=C)
            nc.vector.tensor_mul(
                wp3[:, 0:CSPLIT],
                w3[:, 0:CSPLIT],
                c_mul[:, 0:CSPLIT].unsqueeze(2).broadcast_to(
                    (RS, CSPLIT, O * I)
                ),
            )
            nc.gpsimd.tensor_mul(
                wp3[:, CSPLIT:C],
                w3[:, CSPLIT:C],
                c_mul[:, CSPLIT:C].unsqueeze(2).broadcast_to(
                    (RS, C - CSPLIT, O * I)
                ),
            )
        else:
            # final iteration: ReduceScatter, local squash on batch shard
            if fake_cc:
                src_rs = cc_in[t][0:BS, :]
            else:
                nc.gpsimd.collective_compute(
                    "ReduceScatter",
                    mybir.AluOpType.add,
                    replica_groups=rg,
                    ins=[cc_in[t][:].opt()],
                    outs=[rs_out[:].opt()],
                )
                src_rs = rs_out[:]
            sf = pool.tile([2 * BS, CO // 2], CCDT)
            vf = pool.tile([2 * BS, CO // 2], F32)
            nc.sync.dma_start(
                sf[:], src_rs.rearrange("b (h c) -> (b h) c", h=2)
            )
            _squash_block(nc, pool, sf, vf, 1, 1.0, f"_{rp}f", groups=C // 2)
            nc.sync.dma_start(
                out_d[:].rearrange("b (h c) -> (b h) c", h=2), vf[:]
            )


_NC_CACHE = {}


def _get_nc():
    if "nc" not in _NC_CACHE:
        _NC_CACHE["nc"] = build_nc()
    return _NC_CACHE["nc"]


def _get_runner():
    """Compile once; reuse the jitted SPMD callable across kernel() calls."""
    if "runner" in _NC_CACHE:
        return _NC_CACHE["runner"]
    import jax
    from jax.sharding import Mesh, PartitionSpec, NamedSharding
    from jax.experimental.shard_map import shard_map
    from concourse import bass2jax

    nc = _get_nc()
    bass2jax.install_neuronx_cc_hook()
    partition_name = (
        nc.partition_id_tensor.name if nc.partition_id_tensor else None
    )
    in_names, out_names, out_avals, zero_outs = [], [], [], []
    for alloc in nc.m.functions[0].allocations:
        if not isinstance(alloc, mybir.MemoryLocationSet):
            continue
        name = alloc.memorylocations[0].name
        if alloc.kind == "ExternalInput":
            if name != partition_name:
                in_names.append(name)
        elif alloc.kind == "ExternalOutput":
            out_names.append(name)
            shape = tuple(alloc.tensor_shape)
            dtype = mybir.dt.np(alloc.dtype)
            out_avals.append(jax.core.ShapedArray(shape, dtype))
            zero_outs.append(np.zeros(shape, dtype))
    n_params = len(in_names)
    n_outs = len(out_avals)
    all_in_names = list(in_names) + list(out_names)
    if partition_name is not None:
        all_in_names.append(partition_name)

    def _body(*args):
        operands = list(args)
        if partition_name is not None:
            operands.append(bass2jax.partition_id_tensor())
        outs = bass2jax._bass_exec_p.bind(
            *operands,
            out_avals=tuple(out_avals),
            in_names=tuple(all_in_names),
            out_names=tuple(out_names),
            lowering_input_output_aliases=(),
            sim_require_finite=True,
            sim_require_nnan=True,
            nc=nc,
        )
        return tuple(outs)

    devices = jax.devices()[:NCORES]
    mesh = Mesh(np.asarray(devices), ("core",))
    in_specs = (PartitionSpec("core"),) * (n_params + n_outs)
    out_specs = (PartitionSpec("core"),) * len(out_names)
    donate = tuple(range(n_params, n_params + n_outs))
    sharded = jax.jit(
        shard_map(_body, mesh=mesh, in_specs=in_specs, out_specs=out_specs,
                  check_rep=False),
        donate_argnums=donate,
        keep_unused=True,
    )

    def run(in_maps):
        concat_in = [
            np.concatenate(
                [np.asarray(in_maps[c][in_names[i]]) for c in range(NCORES)],
                axis=0,
            )
            for i in range(n_params)
        ]
        concat_zeros = [
            np.zeros((NCORES * z.shape[0], *z.shape[1:]), z.dtype)
            for z in zero_outs
        ]
        out_arrs = sharded(*concat_in, *concat_zeros)
        return [
            {
                name: np.asarray(out_arrs[i]).reshape(
                    NCORES, *out_avals[i].shape
                )[c]
                for i, name in enumerate(out_names)
            }
            for c in range(NCORES)
        ]

    _NC_CACHE["runner"] = run
    return run


def make_in_maps(x, W):
    x = np.asarray(x, dtype=np.float32)
    W = np.asarray(W, dtype=np.float32)
    if USE_BF16:
        x = x.astype(np.float16)
        W = W.astype(np.float16)
    in_maps = []
    for k in range(NCORES):
        sl = slice(k * RS, (k + 1) * RS)
        xloc = x[:, :, sl]                     # [B, I, RS]
        xs = np.ascontiguousarray(xloc).reshape(B, I * RS)
        # xt[r, (bc*8+i)*128 + b%128]
        xt = np.ascontiguousarray(
            xloc.reshape(2, 128, I, RS).transpose(3, 0, 2, 1)
        ).reshape(RS, 16 * 128)
        ws = np.ascontiguousarray(W[sl]).reshape(RS, COI)
        in_maps.append({"xs": xs, "ws": ws, "xt": xt})
    return in_maps


def kernel(x, W):
    in_maps = make_in_maps(x, W)
    results = None
    for attempt in range(2):
        try:
            run = _get_runner()
            results = run(in_maps)
            break
        except Exception:
            # Transient device wedges (NRT_EXEC_UNIT_UNRECOVERABLE) have
            # been observed to recover on a fresh attempt; rebuild the
            # compiled runner once before giving up.
            if attempt == 1:
                raise
            _NC_CACHE.clear()
    v = np.concatenate([r["vout"] for r in results], axis=0)
    return v.reshape(B, C, O, 1)


if __name__ == "__main__":
    nc = build_nc()
    print("built ok; instructions:", sum(len(bb.instructions) for bb in nc.main_func.blocks))
